# revision 11
# baseline (speedup 1.0000x reference)
"""Dense GAT layer (N=2048, IN=OUT=512, H=8) on 8 trn2 NeuronCores.

Sharding: 8 cores = 2 i-slabs x 4 head-pairs. Core c: ih=c//4 (output rows
1024*ih..+1024), hp=c%4 (heads 2hp, 2hp+1). The SPMD program is identical on
every core; per-core slab selection is baked into the data by rotating the
node axis j (np.roll) so each core's i-slab is always columns 0:1024 of its
own rotated inputs — the j-sum in softmax/matmul is permutation-invariant.

Math: softmax numerator exp(leaky_relu(s_i+s_j))*mask = max(e^z, e^{0.2z})*mask
with z = s_src[i]+s_dst[j] separable, so each score tile is
  max(A'_i * B_j, D_j) * C_i * mask,  A'=e^{0.8 s_src}, B=e^{s_dst}, D=e^{0.2 s_dst}
and the C_i factor cancels in the softmax division. Per 128x1024 tile that is
one dual-op tensor_scalar (mult+max, per-partition scalars) and one
tensor_tensor mask multiply, both f16 on DVE. Denominators ride along as a
ones column in the stationary Wh operand of the alpha@Wh matmul; the division
is a per-partition tensor_scalar after PE-transposing the output.

Pipelining: x^T is DMA'd in 16 [128,512] sub-chunks and the whole Wh/s/Whaug
prep runs per 512-column span, so the first score tiles start while later
x^T spans are still in flight. All PSUM->SBUF staging copies run on the
otherwise-idle Scalar engine, keeping DVE for the score tiles.
"""

import sys

for _p in ("/opt/trn_rl_repo",):
    if _p not in sys.path:
        sys.path.insert(0, _p)

import numpy as np

import concourse.bacc as bacc
import concourse.mybir as mybir
from concourse import tile
from concourse.bass_utils import run_bass_kernel_spmd

try:  # register the NTFF profile hook shim if the image's antenv lacks it
    import antenv.axon_hooks  # noqa: F401
except ImportError:
    import types

    import antenv

    def _get_hook(_cache={}):
        if "h" not in _cache:
            try:
                from trn_agent_boot.trn_boot import _ntff_profile_via_ctypes
                _cache["h"] = _ntff_profile_via_ctypes("/opt/axon/libaxon_pjrt.so")
            except Exception:
                _cache["h"] = None
        return _cache["h"]

    _m = types.ModuleType("antenv.axon_hooks")
    _m.get_axon_ntff_profile_hook = _get_hook
    _m.set_axon_ntff_profile_hook = lambda h: None
    sys.modules["antenv.axon_hooks"] = _m
    antenv.axon_hooks = _m

N = 2048
C = 512            # in_dim
NT = N // 128      # 16 j-tiles
IS = 1024          # i-slab per core
HPC = 2            # heads per core
DP = 64 * HPC      # 128 cols of W per core
F32 = mybir.dt.float32
F16 = mybir.dt.float16

_cached = {}


def build_program(enable_asserts=True):
    nc = bacc.Bacc("TRN2", target_bir_lowering=False, debug=False,
                   enable_asserts=enable_asserts)
    xT = nc.dram_tensor("xT", [C, N], F32, kind="ExternalInput").ap()
    Wp = nc.dram_tensor("Wp", [C, DP], F32, kind="ExternalInput").ap()
    a_embed = nc.dram_tensor("a_embed", [128, 4], F32, kind="ExternalInput").ap()
    adjT = nc.dram_tensor("adjT", [N, IS], F16, kind="ExternalInput").ap()
    ident = nc.dram_tensor("ident", [128, 128], F32, kind="ExternalInput").ap()
    out = nc.dram_tensor("out", [IS, 2 * 64], F32, kind="ExternalOutput").ap()

    AF = mybir.ActivationFunctionType
    Alu = mybir.AluOpType

    with tile.TileContext(nc) as tc:
        with (
            tc.tile_pool(name="big", bufs=1) as big,
            tc.tile_pool(name="sb", bufs=2) as sb,
            tc.tile_pool(name="wk", bufs=4) as wk,
            tc.tile_pool(name="ps", bufs=3, space="PSUM") as ps,
            tc.tile_pool(name="psacc", bufs=1, space="PSUM") as psacc,
        ):
            # ---- input DMAs (xT split per (span, chunk) so span-0 prep can
            # start after ~1MB; adjT tiles stream behind)
            t_Wp = big.tile([128, 4 * DP], F32, tag="Wp")
            nc.sync.dma_start(t_Wp[:].rearrange("p (c d) -> p c d", c=4),
                              Wp.rearrange("(c p) d -> p c d", p=128))
            t_ae = big.tile([128, 4], F32, tag="ae")
            nc.sync.dma_start(t_ae[:], a_embed)
            t_id = big.tile([128, 128], F32, tag="ident")
            nc.sync.dma_start(t_id[:], ident)
            t_xT = big.tile([128, 4 * N], F32, tag="xT")        # [p, (c, j)]
            xT_v = t_xT[:].rearrange("p (c j) -> p c j", c=4)
            xT_src = xT.rearrange("(c p) j -> p c j", p=128)
            t_adj = big.tile([128, NT * IS], F16, tag="adj")    # [p, (t, i)]
            adj_v = t_adj[:].rearrange("p (t i) -> p t i", t=NT)
            adj_src = adjT.rearrange("(t p) i -> p t i", p=128)
            for s in range(2):
                nc.sync.dma_start(xT_v[:, :, 512 * s:512 * (s + 1)],
                                  xT_src[:, :, 512 * s:512 * (s + 1)])
            nc.sync.dma_start(adj_v[:, 0:8, :], adj_src[:, 0:8, :])
            for s in range(2, 4):
                nc.sync.dma_start(xT_v[:, :, 512 * s:512 * (s + 1)],
                                  xT_src[:, :, 512 * s:512 * (s + 1)])
            nc.sync.dma_start(adj_v[:, 8:16, :], adj_src[:, 8:16, :])
            Wp_v = t_Wp[:].rearrange("p (c d) -> p c d", c=4)

            t_WhT = big.tile([128, N], F32, tag="WhT")
            t_s = big.tile([4, N], F32, tag="s")
            t_B = big.tile([128, 2 * NT], F32, tag="B")
            t_D = big.tile([128, 2 * NT], F32, tag="D")
            t_Wa = big.tile([128, NT * 130], F16, tag="Wa")
            nc.vector.memset(t_Wa[:, 64:NT * 130:130], 1.0)
            nc.vector.memset(t_Wa[:, 129:NT * 130:130], 1.0)
            t_Ab = big.tile([128, HPC * IS], F16, tag="Ab")
            t_arow = sb.tile([1, HPC * IS], F16, tag="arow")

            # ---- per-span prep: Wh^T, s, B/D scalars, Whaug j-tiles ----
            for s in range(4):
                sp = slice(512 * s, 512 * (s + 1))
                p_wht = ps.tile([128, 512], F32, tag="work", name="p_wht")
                for ct in range(4):
                    nc.tensor.matmul(p_wht[:], Wp_v[:, ct, :],
                                     xT_v[:, ct, sp],
                                     start=(ct == 0), stop=(ct == 3))
                nc.scalar.copy(t_WhT[:, sp], p_wht[:])
                if s < 2:
                    # A' row chunk (slab s_src) — critical path for the score
                    # tiles, so it goes first: K=128/M=1 matvec + exp
                    for k in range(HPC):
                        p_row = ps.tile([128, 512], F32, tag="work", name="p_row")
                        nc.tensor.matmul(p_row[0:1, :], t_ae[:, 2 * k:2 * k + 1],
                                         t_WhT[:, sp], start=True, stop=True)
                        nc.scalar.activation(
                            t_arow[0:1, IS * k + 512 * s:IS * k + 512 * (s + 1)],
                            p_row[0:1, :], AF.Exp, scale=0.8)
                if s == 1:
                    for k in range(HPC):
                        nc.gpsimd.partition_broadcast(
                            t_Ab[:, IS * k:IS * (k + 1)],
                            t_arow[0:1, IS * k:IS * (k + 1)])
                # s rows for this span
                p_s = ps.tile([128, 512], F32, tag="work", name="p_s")
                nc.tensor.matmul(p_s[0:4, :], t_ae[:], t_WhT[:, sp],
                                 start=True, stop=True)
                nc.scalar.copy(t_s[:, sp], p_s[0:4, :])
                # B/D per-partition scalars for the 4 j-tiles of this span
                p_sT = ps.tile([128, 512], F32, tag="work", name="p_sT")
                for tt in range(4):
                    t = 4 * s + tt
                    nc.tensor.transpose(p_sT[:, 4 * tt:4 * tt + 4],
                                        t_s[:, 128 * t:128 * (t + 1)],
                                        t_id[0:4, 0:4])
                for k in range(HPC):
                    nc.scalar.activation(t_B[:, NT * k + 4 * s:NT * k + 4 * (s + 1)],
                                         p_sT[:, (2 * k + 1):16:4], AF.Exp)
                    nc.scalar.activation(t_D[:, NT * k + 4 * s:NT * k + 4 * (s + 1)],
                                         p_sT[:, (2 * k + 1):16:4], AF.Exp, scale=0.2)
                # Whaug tiles for this span
                for tt in range(4):
                    t = 4 * s + tt
                    p_wh = ps.tile([128, 512], F32, tag="work", name="p_wh")
                    nc.tensor.transpose(p_wh[:, 0:128],
                                        t_WhT[:, 128 * t:128 * (t + 1)], t_id[:])
                    dst = t_Wa[:, 130 * t:130 * t + 130].rearrange(
                        "p (g r) -> p g r", r=65)[:, :, 0:64]
                    nc.scalar.copy(dst,
                                   p_wh[:, 0:128].rearrange("p (g r) -> p g r", r=64))

            # ---- main loop: k-outer so head-0 fixup overlaps head-1 sweep
            p_acc = [psacc.tile([65, IS], F32, tag=f"acc{k}", name=f"p_acc{k}")
                     for k in range(HPC)]
            for k in range(HPC):
                for t in range(NT):
                    t_u = wk.tile([128, IS], F16, tag="u")
                    nc.vector.tensor_scalar(
                        t_u[:], t_Ab[:, IS * k:IS * (k + 1)],
                        t_B[:, NT * k + t:NT * k + t + 1],
                        t_D[:, NT * k + t:NT * k + t + 1], Alu.mult, Alu.max)
                    t_p = wk.tile([128, IS], F16, tag="p")
                    nc.vector.tensor_tensor(t_p[:], t_u[:],
                                            t_adj[:, IS * t:IS * (t + 1)], Alu.mult)
                    for sp2 in range(IS // 512):
                        nc.tensor.matmul(
                            p_acc[k][:, 512 * sp2:512 * (sp2 + 1)],
                            t_Wa[:, 130 * t + 65 * k:130 * t + 65 * (k + 1)],
                            t_p[:, 512 * sp2:512 * (sp2 + 1)],
                            start=(t == 0), stop=(t == NT - 1))
                # fixup: divide by denominator, emit [i, (k, d)]
                t_oc = sb.tile([65, IS], F32, tag="oc")
                nc.scalar.copy(t_oc[:], p_acc[k][:])
                t_rcp = sb.tile([128, 8], F32, tag="rcp")
                t_of = sb.tile([128, 8 * 64], F32, tag="of")
                for q in range(8):
                    p_oT = ps.tile([128, 512], F32, tag="work", name="p_oT")
                    nc.tensor.transpose(p_oT[:, 0:65],
                                        t_oc[:, 128 * q:128 * (q + 1)],
                                        t_id[0:65, 0:65])
                    nc.vector.reciprocal(t_rcp[:, q:q + 1], p_oT[:, 64:65])
                    nc.vector.tensor_scalar(
                        t_of[:, 64 * q:64 * (q + 1)], p_oT[:, 0:64],
                        t_rcp[:, q:q + 1], None, Alu.mult)
                nc.sync.dma_start(
                    out.rearrange("(q p) c -> p q c", p=128)[:, :, 64 * k:64 * (k + 1)],
                    t_of[:].rearrange("p (q d) -> p q d", q=8))
    nc.compile()
    return nc


def make_in_maps(x, adj, W, a):
    x = np.asarray(x, np.float32)
    adj = np.asarray(adj)
    W = np.asarray(W, np.float32)
    a = np.asarray(a, np.float32)
    xT = np.ascontiguousarray(x.T)
    a_embed = np.zeros((128, 4), np.float32)
    a_embed[0:64, 0] = a[0:64]      # head k=0 src
    a_embed[0:64, 1] = a[64:128]    # head k=0 dst
    a_embed[64:128, 2] = a[0:64]    # head k=1 src
    a_embed[64:128, 3] = a[64:128]  # head k=1 dst
    ident = np.eye(128, dtype=np.float32)
    adjT_f16 = [np.ascontiguousarray(adj[IS * ih:IS * (ih + 1), :].T)
                .astype(np.float16) for ih in range(2)]
    in_maps = []
    for c in range(8):
        ih, hp = c // 4, c % 4
        in_maps.append({
            "xT": np.roll(xT, -IS * ih, axis=1) if ih else xT,
            "Wp": np.ascontiguousarray(W[:, 128 * hp:128 * (hp + 1)]),
            "a_embed": a_embed,
            "adjT": np.roll(adjT_f16[ih], -IS * ih, axis=0) if ih else adjT_f16[ih],
            "ident": ident,
        })
    return in_maps


def kernel(x, adj, W, a, trace=False, **trace_kw):
    if "nc" not in _cached:
        _cached["nc"] = build_program()
    nc = _cached["nc"]
    in_maps = make_in_maps(x, adj, W, a)
    res = run_bass_kernel_spmd(nc, in_maps, core_ids=list(range(8)),
                               trace=trace, **trace_kw)
    out = np.empty((N, 512), np.float32)
    for c in range(8):
        ih, hp = c // 4, c % 4
        o = res.results[c]["out"]           # [IS, 128] = [i_local, (k, d)]
        out[IS * ih:IS * (ih + 1), 128 * hp:128 * (hp + 1)] = o
    if trace:
        kernel.last_result = res
    return out


# revision 12
# speedup vs baseline: 1.0142x; 1.0142x over previous
"""Dense GAT layer (N=2048, IN=OUT=512, H=8) on 8 trn2 NeuronCores.

Sharding: 8 cores = 2 i-slabs x 4 head-pairs. Core c: ih=c//4 (output rows
1024*ih..+1024), hp=c%4 (heads 2hp, 2hp+1). The SPMD program is identical on
every core; per-core slab selection is baked into the data by rotating the
node axis j (np.roll) so each core's i-slab is always columns 0:1024 of its
own rotated inputs — the j-sum in softmax/matmul is permutation-invariant.

Math: softmax numerator exp(leaky_relu(s_i+s_j))*mask = max(e^z, e^{0.2z})*mask
with z = s_src[i]+s_dst[j] separable, so each score tile is
  max(A'_i * B_j, D_j) * C_i * mask,  A'=e^{0.8 s_src}, B=e^{s_dst}, D=e^{0.2 s_dst}
and the C_i factor cancels in the softmax division. Per 128x1024 tile that is
one dual-op tensor_scalar (mult+max, per-partition scalars) and one
tensor_tensor mask multiply, both f16 on DVE. Denominators ride along as a
ones column in the stationary Wh operand of the alpha@Wh matmul; the division
is a per-partition tensor_scalar after PE-transposing the output.

Pipelining: x^T is DMA'd in 16 [128,512] sub-chunks and the whole Wh/s/Whaug
prep runs per 512-column span, so the first score tiles start while later
x^T spans are still in flight. All PSUM->SBUF staging copies run on the
otherwise-idle Scalar engine, keeping DVE for the score tiles.
"""

import sys

for _p in ("/opt/trn_rl_repo",):
    if _p not in sys.path:
        sys.path.insert(0, _p)

import numpy as np

import concourse.bacc as bacc
import concourse.mybir as mybir
from concourse import tile
from concourse.bass_utils import run_bass_kernel_spmd

try:  # register the NTFF profile hook shim if the image's antenv lacks it
    import antenv.axon_hooks  # noqa: F401
except ImportError:
    import types

    import antenv

    def _get_hook(_cache={}):
        if "h" not in _cache:
            try:
                from trn_agent_boot.trn_boot import _ntff_profile_via_ctypes
                _cache["h"] = _ntff_profile_via_ctypes("/opt/axon/libaxon_pjrt.so")
            except Exception:
                _cache["h"] = None
        return _cache["h"]

    _m = types.ModuleType("antenv.axon_hooks")
    _m.get_axon_ntff_profile_hook = _get_hook
    _m.set_axon_ntff_profile_hook = lambda h: None
    sys.modules["antenv.axon_hooks"] = _m
    antenv.axon_hooks = _m

N = 2048
C = 512            # in_dim
NT = N // 128      # 16 j-tiles
IS = 1024          # i-slab per core
HPC = 2            # heads per core
DP = 64 * HPC      # 128 cols of W per core
F32 = mybir.dt.float32
F16 = mybir.dt.float16

_cached = {}


def build_program(enable_asserts=True):
    nc = bacc.Bacc("TRN2", target_bir_lowering=False, debug=False,
                   enable_asserts=enable_asserts)
    xT = nc.dram_tensor("xT", [C, N], F32, kind="ExternalInput").ap()
    Wp = nc.dram_tensor("Wp", [C, DP], F32, kind="ExternalInput").ap()
    a_embed = nc.dram_tensor("a_embed", [128, 4], F32, kind="ExternalInput").ap()
    adjT = nc.dram_tensor("adjT", [N, IS], F16, kind="ExternalInput").ap()
    ident = nc.dram_tensor("ident", [128, 128], F32, kind="ExternalInput").ap()
    out = nc.dram_tensor("out", [IS, 2 * 64], F32, kind="ExternalOutput").ap()

    AF = mybir.ActivationFunctionType
    Alu = mybir.AluOpType

    with tile.TileContext(nc) as tc:
        with (
            tc.tile_pool(name="big", bufs=1) as big,
            tc.tile_pool(name="sb", bufs=2) as sb,
            tc.tile_pool(name="wk", bufs=6) as wk,
            tc.tile_pool(name="ps", bufs=3, space="PSUM") as ps,
            tc.tile_pool(name="psacc", bufs=1, space="PSUM") as psacc,
        ):
            # ---- input DMAs (xT split per (span, chunk) so span-0 prep can
            # start after ~1MB; adjT tiles stream behind)
            t_Wp = big.tile([128, 4 * DP], F32, tag="Wp")
            nc.sync.dma_start(t_Wp[:].rearrange("p (c d) -> p c d", c=4),
                              Wp.rearrange("(c p) d -> p c d", p=128))
            t_ae = big.tile([128, 4], F32, tag="ae")
            nc.sync.dma_start(t_ae[:], a_embed)
            t_id = big.tile([128, 128], F32, tag="ident")
            nc.sync.dma_start(t_id[:], ident)
            t_xT = big.tile([128, 4 * N], F32, tag="xT")        # [p, (c, j)]
            xT_v = t_xT[:].rearrange("p (c j) -> p c j", c=4)
            xT_src = xT.rearrange("(c p) j -> p c j", p=128)
            t_adj = big.tile([128, NT * IS], F16, tag="adj")    # [p, (t, i)]
            adj_v = t_adj[:].rearrange("p (t i) -> p t i", t=NT)
            adj_src = adjT.rearrange("(t p) i -> p t i", p=128)
            for s in range(2):
                nc.sync.dma_start(xT_v[:, :, 512 * s:512 * (s + 1)],
                                  xT_src[:, :, 512 * s:512 * (s + 1)])
            nc.sync.dma_start(adj_v[:, 0:8, :], adj_src[:, 0:8, :])
            for s in range(2, 4):
                nc.sync.dma_start(xT_v[:, :, 512 * s:512 * (s + 1)],
                                  xT_src[:, :, 512 * s:512 * (s + 1)])
            nc.sync.dma_start(adj_v[:, 8:16, :], adj_src[:, 8:16, :])
            Wp_v = t_Wp[:].rearrange("p (c d) -> p c d", c=4)

            t_WhT = big.tile([128, N], F32, tag="WhT")
            t_s = big.tile([4, N], F32, tag="s")
            t_B = big.tile([128, 2 * NT], F32, tag="B")
            t_D = big.tile([128, 2 * NT], F32, tag="D")
            t_Wa = big.tile([128, NT * 130], F16, tag="Wa")
            nc.vector.memset(t_Wa[:, 64:NT * 130:130], 1.0)
            nc.vector.memset(t_Wa[:, 129:NT * 130:130], 1.0)
            t_Ab = big.tile([128, HPC * IS], F16, tag="Ab")
            t_arow = sb.tile([1, HPC * IS], F16, tag="arow")

            # ---- stage 0: Ab critical chain (Wh^T spans 0-1 -> A' -> bcast)
            for s in range(2):
                sp = slice(512 * s, 512 * (s + 1))
                p_wht = ps.tile([128, 512], F32, tag="work", name="p_wht0")
                for ct in range(4):
                    nc.tensor.matmul(p_wht[:], Wp_v[:, ct, :],
                                     xT_v[:, ct, sp],
                                     start=(ct == 0), stop=(ct == 3))
                nc.scalar.copy(t_WhT[:, sp], p_wht[:])
                for k in range(HPC):
                    p_row = ps.tile([128, 512], F32, tag="work", name="p_row")
                    nc.tensor.matmul(p_row[0:1, :], t_ae[:, 2 * k:2 * k + 1],
                                     t_WhT[:, sp], start=True, stop=True)
                    nc.scalar.activation(
                        t_arow[0:1, IS * k + 512 * s:IS * k + 512 * (s + 1)],
                        p_row[0:1, :], AF.Exp, scale=0.8)
            for k in range(HPC):
                nc.gpsimd.partition_broadcast(t_Ab[:, IS * k:IS * (k + 1)],
                                              t_arow[0:1, IS * k:IS * (k + 1)])

            # ---- per-span prep: Wh^T (spans 2-3), s, B/D scalars, Whaug ----
            for s in range(4):
                sp = slice(512 * s, 512 * (s + 1))
                if s >= 2:
                    p_wht = ps.tile([128, 512], F32, tag="work", name="p_wht")
                    for ct in range(4):
                        nc.tensor.matmul(p_wht[:], Wp_v[:, ct, :],
                                         xT_v[:, ct, sp],
                                         start=(ct == 0), stop=(ct == 3))
                    nc.scalar.copy(t_WhT[:, sp], p_wht[:])
                # s rows for this span
                p_s = ps.tile([128, 512], F32, tag="work", name="p_s")
                nc.tensor.matmul(p_s[0:4, :], t_ae[:], t_WhT[:, sp],
                                 start=True, stop=True)
                nc.scalar.copy(t_s[:, sp], p_s[0:4, :])
                # B/D per-partition scalars for the 4 j-tiles of this span
                p_sT = ps.tile([128, 512], F32, tag="work", name="p_sT")
                for tt in range(4):
                    t = 4 * s + tt
                    nc.tensor.transpose(p_sT[:, 4 * tt:4 * tt + 4],
                                        t_s[:, 128 * t:128 * (t + 1)],
                                        t_id[0:4, 0:4])
                for k in range(HPC):
                    nc.scalar.activation(t_B[:, NT * k + 4 * s:NT * k + 4 * (s + 1)],
                                         p_sT[:, (2 * k + 1):16:4], AF.Exp)
                    nc.scalar.activation(t_D[:, NT * k + 4 * s:NT * k + 4 * (s + 1)],
                                         p_sT[:, (2 * k + 1):16:4], AF.Exp, scale=0.2)
                # Whaug tiles for this span
                for tt in range(4):
                    t = 4 * s + tt
                    p_wh = ps.tile([128, 512], F32, tag="work", name="p_wh")
                    nc.tensor.transpose(p_wh[:, 0:128],
                                        t_WhT[:, 128 * t:128 * (t + 1)], t_id[:])
                    dst = t_Wa[:, 130 * t:130 * t + 130].rearrange(
                        "p (g r) -> p g r", r=65)[:, :, 0:64]
                    nc.scalar.copy(dst,
                                   p_wh[:, 0:128].rearrange("p (g r) -> p g r", r=64))

            # ---- main loop: k-outer so head-0 fixup overlaps head-1 sweep
            p_acc = [psacc.tile([65, IS], F32, tag=f"acc{k}", name=f"p_acc{k}")
                     for k in range(HPC)]
            for k in range(HPC):
                for t in range(NT):
                    t_u = wk.tile([128, IS], F16, tag="u")
                    nc.vector.tensor_scalar(
                        t_u[:], t_Ab[:, IS * k:IS * (k + 1)],
                        t_B[:, NT * k + t:NT * k + t + 1],
                        t_D[:, NT * k + t:NT * k + t + 1], Alu.mult, Alu.max)
                    t_p = wk.tile([128, IS], F16, tag="p")
                    nc.vector.tensor_tensor(t_p[:], t_u[:],
                                            t_adj[:, IS * t:IS * (t + 1)], Alu.mult)
                    for sp2 in range(IS // 512):
                        nc.tensor.matmul(
                            p_acc[k][:, 512 * sp2:512 * (sp2 + 1)],
                            t_Wa[:, 130 * t + 65 * k:130 * t + 65 * (k + 1)],
                            t_p[:, 512 * sp2:512 * (sp2 + 1)],
                            start=(t == 0), stop=(t == NT - 1))
                # fixup: divide by denominator, emit [i, (k, d)]
                t_oc = sb.tile([65, IS], F32, tag="oc")
                nc.scalar.copy(t_oc[:], p_acc[k][:])
                t_rcp = sb.tile([128, 8], F32, tag="rcp")
                t_of = sb.tile([128, 8 * 64], F32, tag="of")
                for q in range(8):
                    p_oT = ps.tile([128, 512], F32, tag="work", name="p_oT")
                    nc.tensor.transpose(p_oT[:, 0:65],
                                        t_oc[:, 128 * q:128 * (q + 1)],
                                        t_id[0:65, 0:65])
                    nc.vector.reciprocal(t_rcp[:, q:q + 1], p_oT[:, 64:65])
                    nc.scalar.activation(
                        t_of[:, 64 * q:64 * (q + 1)], p_oT[:, 0:64],
                        AF.Identity, scale=t_rcp[:, q:q + 1])
                nc.sync.dma_start(
                    out.rearrange("(q p) c -> p q c", p=128)[:, :, 64 * k:64 * (k + 1)],
                    t_of[:].rearrange("p (q d) -> p q d", q=8))
    nc.compile()
    return nc


def make_in_maps(x, adj, W, a):
    x = np.asarray(x, np.float32)
    adj = np.asarray(adj)
    W = np.asarray(W, np.float32)
    a = np.asarray(a, np.float32)
    xT = np.ascontiguousarray(x.T)
    a_embed = np.zeros((128, 4), np.float32)
    a_embed[0:64, 0] = a[0:64]      # head k=0 src
    a_embed[0:64, 1] = a[64:128]    # head k=0 dst
    a_embed[64:128, 2] = a[0:64]    # head k=1 src
    a_embed[64:128, 3] = a[64:128]  # head k=1 dst
    ident = np.eye(128, dtype=np.float32)
    adjT_f16 = [np.ascontiguousarray(adj[IS * ih:IS * (ih + 1), :].T)
                .astype(np.float16) for ih in range(2)]
    in_maps = []
    for c in range(8):
        ih, hp = c // 4, c % 4
        in_maps.append({
            "xT": np.roll(xT, -IS * ih, axis=1) if ih else xT,
            "Wp": np.ascontiguousarray(W[:, 128 * hp:128 * (hp + 1)]),
            "a_embed": a_embed,
            "adjT": np.roll(adjT_f16[ih], -IS * ih, axis=0) if ih else adjT_f16[ih],
            "ident": ident,
        })
    return in_maps


def kernel(x, adj, W, a, trace=False, **trace_kw):
    if "nc" not in _cached:
        _cached["nc"] = build_program()
    nc = _cached["nc"]
    in_maps = make_in_maps(x, adj, W, a)
    res = run_bass_kernel_spmd(nc, in_maps, core_ids=list(range(8)),
                               trace=trace, **trace_kw)
    out = np.empty((N, 512), np.float32)
    for c in range(8):
        ih, hp = c // 4, c % 4
        o = res.results[c]["out"]           # [IS, 128] = [i_local, (k, d)]
        out[IS * ih:IS * (ih + 1), 128 * hp:128 * (hp + 1)] = o
    if trace:
        kernel.last_result = res
    return out


# revision 13
# speedup vs baseline: 1.2358x; 1.2185x over previous
"""Dense GAT layer (N=2048, IN=OUT=512, H=8) on 8 trn2 NeuronCores.

Sharding: 8 cores = 2 i-slabs x 4 head-pairs. Core c: ih=c//4 (output rows
1024*ih..+1024), hp=c%4 (heads 2hp, 2hp+1). The SPMD program is identical on
every core; per-core slab selection is baked into the data by rotating the
node axis j (np.roll) so each core's i-slab is always columns 0:1024 of its
own rotated inputs — the j-sum in softmax/matmul is permutation-invariant.

Math: softmax numerator exp(leaky_relu(s_i+s_j))*mask = max(e^z, e^{0.2z})*mask
with z = s_src[i]+s_dst[j] separable, so each score tile is
  max(A'_i * B_j, D_j) * C_i * mask,  A'=e^{0.8 s_src}, B=e^{s_dst}, D=e^{0.2 s_dst}
and the C_i factor cancels in the softmax division. Per 128x1024 tile that is
one dual-op tensor_scalar (mult+max, per-partition scalars) and one
tensor_tensor mask multiply, both f16 on DVE. Denominators ride along as a
ones column in the stationary Wh operand of the alpha@Wh matmul; the division
is a per-partition tensor_scalar after PE-transposing the output.

Pipelining: x^T is DMA'd in 16 [128,512] sub-chunks and the whole Wh/s/Whaug
prep runs per 512-column span, so the first score tiles start while later
x^T spans are still in flight. All PSUM->SBUF staging copies run on the
otherwise-idle Scalar engine, keeping DVE for the score tiles.
"""

import sys

for _p in ("/opt/trn_rl_repo",):
    if _p not in sys.path:
        sys.path.insert(0, _p)

import numpy as np

import concourse.bacc as bacc
import concourse.mybir as mybir
from concourse import tile
from concourse.bass_utils import run_bass_kernel_spmd

try:  # register the NTFF profile hook shim if the image's antenv lacks it
    import antenv.axon_hooks  # noqa: F401
except ImportError:
    import types

    import antenv

    def _get_hook(_cache={}):
        if "h" not in _cache:
            try:
                from trn_agent_boot.trn_boot import _ntff_profile_via_ctypes
                _cache["h"] = _ntff_profile_via_ctypes("/opt/axon/libaxon_pjrt.so")
            except Exception:
                _cache["h"] = None
        return _cache["h"]

    _m = types.ModuleType("antenv.axon_hooks")
    _m.get_axon_ntff_profile_hook = _get_hook
    _m.set_axon_ntff_profile_hook = lambda h: None
    sys.modules["antenv.axon_hooks"] = _m
    antenv.axon_hooks = _m

N = 2048
C = 512            # in_dim
NT = N // 128      # 16 j-tiles
IS = 1024          # i-slab per core
HPC = 2            # heads per core
DP = 64 * HPC      # 128 cols of W per core
F32 = mybir.dt.float32
F16 = mybir.dt.float16

_cached = {}


def build_program(enable_asserts=True):
    nc = bacc.Bacc("TRN2", target_bir_lowering=False, debug=False,
                   enable_asserts=enable_asserts)
    xT = nc.dram_tensor("xT", [C, N], F16, kind="ExternalInput").ap()
    Wp = nc.dram_tensor("Wp", [C, DP], F16, kind="ExternalInput").ap()
    a_embed = nc.dram_tensor("a_embed", [128, 4], F16, kind="ExternalInput").ap()
    adjT = nc.dram_tensor("adjT", [N, IS], F16, kind="ExternalInput").ap()
    ident = nc.dram_tensor("ident", [128, 128], F32, kind="ExternalInput").ap()
    out = nc.dram_tensor("out", [IS, 2 * 64], F32, kind="ExternalOutput").ap()

    AF = mybir.ActivationFunctionType
    Alu = mybir.AluOpType

    with tile.TileContext(nc) as tc:
        with (
            tc.tile_pool(name="big", bufs=1) as big,
            tc.tile_pool(name="sb", bufs=2) as sb,
            tc.tile_pool(name="wk", bufs=6) as wk,
            tc.tile_pool(name="ps", bufs=3, space="PSUM") as ps,
            tc.tile_pool(name="psacc", bufs=1, space="PSUM") as psacc,
        ):
            # ---- input DMAs (xT split per (span, chunk) so span-0 prep can
            # start after ~1MB; adjT tiles stream behind)
            t_Wp = big.tile([128, 4 * DP], F16, tag="Wp")
            nc.sync.dma_start(t_Wp[:].rearrange("p (c d) -> p c d", c=4),
                              Wp.rearrange("(c p) d -> p c d", p=128))
            t_ae = big.tile([128, 4], F16, tag="ae")
            nc.sync.dma_start(t_ae[:], a_embed)
            t_id = big.tile([128, 128], F32, tag="ident")
            nc.sync.dma_start(t_id[:], ident)
            t_id16 = big.tile([128, 128], F16, tag="ident16")
            nc.scalar.copy(t_id16[:], t_id[:])
            t_xT = big.tile([128, 4 * N], F16, tag="xT")        # [p, (c, j)]
            xT_v = t_xT[:].rearrange("p (c j) -> p c j", c=4)
            xT_src = xT.rearrange("(c p) j -> p c j", p=128)
            t_adj = big.tile([128, NT * IS], F16, tag="adj")    # [p, (t, i)]
            adj_v = t_adj[:].rearrange("p (t i) -> p t i", t=NT)
            adj_src = adjT.rearrange("(t p) i -> p t i", p=128)
            for s in range(2):
                nc.sync.dma_start(xT_v[:, :, 512 * s:512 * (s + 1)],
                                  xT_src[:, :, 512 * s:512 * (s + 1)])
            nc.sync.dma_start(adj_v[:, 0:8, :], adj_src[:, 0:8, :])
            for s in range(2, 4):
                nc.sync.dma_start(xT_v[:, :, 512 * s:512 * (s + 1)],
                                  xT_src[:, :, 512 * s:512 * (s + 1)])
            nc.sync.dma_start(adj_v[:, 8:16, :], adj_src[:, 8:16, :])
            Wp_v = t_Wp[:].rearrange("p (c d) -> p c d", c=4)

            t_WhT = big.tile([128, N], F16, tag="WhT")
            t_s = big.tile([4, N], F16, tag="s")
            t_B = big.tile([128, 2 * NT], F32, tag="B")
            t_D = big.tile([128, 2 * NT], F32, tag="D")
            t_Wa = big.tile([128, NT * 130], F16, tag="Wa")
            nc.vector.memset(t_Wa[:, 64:NT * 130:130], 1.0)
            nc.vector.memset(t_Wa[:, 129:NT * 130:130], 1.0)
            t_Ab = big.tile([128, HPC * IS], F16, tag="Ab")
            t_arow = sb.tile([1, HPC * IS], F16, tag="arow")

            # ---- stage 0: Ab critical chain (Wh^T spans 0-1 -> A' -> bcast)
            for s in range(2):
                sp = slice(512 * s, 512 * (s + 1))
                p_wht = ps.tile([128, 512], F32, tag="work", name="p_wht0")
                for ct in range(4):
                    nc.tensor.matmul(p_wht[:], Wp_v[:, ct, :],
                                     xT_v[:, ct, sp],
                                     start=(ct == 0), stop=(ct == 3))
                nc.scalar.copy(t_WhT[:, sp], p_wht[:])
                for k in range(HPC):
                    p_row = ps.tile([128, 512], F32, tag="work", name="p_row")
                    nc.tensor.matmul(p_row[0:1, :], t_ae[:, 2 * k:2 * k + 1],
                                     t_WhT[:, sp], start=True, stop=True)
                    nc.scalar.activation(
                        t_arow[0:1, IS * k + 512 * s:IS * k + 512 * (s + 1)],
                        p_row[0:1, :], AF.Exp, scale=0.8)
            for k in range(HPC):
                nc.gpsimd.partition_broadcast(t_Ab[:, IS * k:IS * (k + 1)],
                                              t_arow[0:1, IS * k:IS * (k + 1)])

            # ---- per-span prep: Wh^T (spans 2-3), s, B/D scalars, Whaug ----
            for s in range(4):
                sp = slice(512 * s, 512 * (s + 1))
                if s >= 2:
                    p_wht = ps.tile([128, 512], F32, tag="work", name="p_wht")
                    for ct in range(4):
                        nc.tensor.matmul(p_wht[:], Wp_v[:, ct, :],
                                         xT_v[:, ct, sp],
                                         start=(ct == 0), stop=(ct == 3))
                    nc.scalar.copy(t_WhT[:, sp], p_wht[:])
                # s rows for this span
                p_s = ps.tile([128, 512], F32, tag="work", name="p_s")
                nc.tensor.matmul(p_s[0:4, :], t_ae[:], t_WhT[:, sp],
                                 start=True, stop=True)
                nc.scalar.copy(t_s[:, sp], p_s[0:4, :])
                # B/D per-partition scalars for the 4 j-tiles of this span
                p_sT = ps.tile([128, 512], F16, tag="work", name="p_sT")
                for tt in range(4):
                    t = 4 * s + tt
                    nc.tensor.transpose(p_sT[:, 4 * tt:4 * tt + 4],
                                        t_s[:, 128 * t:128 * (t + 1)],
                                        t_id16[0:4, 0:4])
                for k in range(HPC):
                    nc.scalar.activation(t_B[:, NT * k + 4 * s:NT * k + 4 * (s + 1)],
                                         p_sT[:, (2 * k + 1):16:4], AF.Exp)
                    nc.scalar.activation(t_D[:, NT * k + 4 * s:NT * k + 4 * (s + 1)],
                                         p_sT[:, (2 * k + 1):16:4], AF.Exp, scale=0.2)
                # Whaug tiles for this span
                for tt in range(4):
                    t = 4 * s + tt
                    p_wh = ps.tile([128, 512], F16, tag="work", name="p_wh")
                    nc.tensor.transpose(p_wh[:, 0:128],
                                        t_WhT[:, 128 * t:128 * (t + 1)], t_id16[:])
                    dst = t_Wa[:, 130 * t:130 * t + 130].rearrange(
                        "p (g r) -> p g r", r=65)[:, :, 0:64]
                    nc.scalar.copy(dst,
                                   p_wh[:, 0:128].rearrange("p (g r) -> p g r", r=64))

            # ---- main loop: k-outer so head-0 fixup overlaps head-1 sweep
            p_acc = [psacc.tile([65, IS], F32, tag=f"acc{k}", name=f"p_acc{k}")
                     for k in range(HPC)]
            for k in range(HPC):
                for t in range(NT):
                    t_u = wk.tile([128, IS], F16, tag="u")
                    nc.vector.tensor_scalar(
                        t_u[:], t_Ab[:, IS * k:IS * (k + 1)],
                        t_B[:, NT * k + t:NT * k + t + 1],
                        t_D[:, NT * k + t:NT * k + t + 1], Alu.mult, Alu.max)
                    t_p = wk.tile([128, IS], F16, tag="p")
                    nc.vector.tensor_tensor(t_p[:], t_u[:],
                                            t_adj[:, IS * t:IS * (t + 1)], Alu.mult)
                    for sp2 in range(IS // 512):
                        nc.tensor.matmul(
                            p_acc[k][:, 512 * sp2:512 * (sp2 + 1)],
                            t_Wa[:, 130 * t + 65 * k:130 * t + 65 * (k + 1)],
                            t_p[:, 512 * sp2:512 * (sp2 + 1)],
                            start=(t == 0), stop=(t == NT - 1))
                # fixup: divide by denominator, emit [i, (k, d)]
                t_oc = sb.tile([65, IS], F32, tag="oc")
                nc.scalar.copy(t_oc[:], p_acc[k][:])
                t_rcp = sb.tile([128, 8], F32, tag="rcp")
                t_of = sb.tile([128, 8 * 64], F32, tag="of")
                for q in range(8):
                    p_oT = ps.tile([128, 512], F32, tag="work", name="p_oT")
                    nc.tensor.transpose(p_oT[:, 0:65],
                                        t_oc[:, 128 * q:128 * (q + 1)],
                                        t_id[0:65, 0:65])
                    nc.vector.reciprocal(t_rcp[:, q:q + 1], p_oT[:, 64:65])
                    nc.scalar.activation(
                        t_of[:, 64 * q:64 * (q + 1)], p_oT[:, 0:64],
                        AF.Identity, scale=t_rcp[:, q:q + 1])
                nc.sync.dma_start(
                    out.rearrange("(q p) c -> p q c", p=128)[:, :, 64 * k:64 * (k + 1)],
                    t_of[:].rearrange("p (q d) -> p q d", q=8))
    nc.compile()
    return nc


def make_in_maps(x, adj, W, a):
    x = np.asarray(x, np.float32)
    adj = np.asarray(adj)
    W = np.asarray(W, np.float32)
    a = np.asarray(a, np.float32)
    xT = np.ascontiguousarray(x.T).astype(np.float16)
    a_embed = np.zeros((128, 4), np.float16)
    a_embed[0:64, 0] = a[0:64]      # head k=0 src
    a_embed[0:64, 1] = a[64:128]    # head k=0 dst
    a_embed[64:128, 2] = a[0:64]    # head k=1 src
    a_embed[64:128, 3] = a[64:128]  # head k=1 dst
    ident = np.eye(128, dtype=np.float32)
    adjT_f16 = [np.ascontiguousarray(adj[IS * ih:IS * (ih + 1), :].T)
                .astype(np.float16) for ih in range(2)]
    in_maps = []
    for c in range(8):
        ih, hp = c // 4, c % 4
        in_maps.append({
            "xT": np.roll(xT, -IS * ih, axis=1) if ih else xT,
            "Wp": np.ascontiguousarray(
                W[:, 128 * hp:128 * (hp + 1)]).astype(np.float16),
            "a_embed": a_embed,
            "adjT": np.roll(adjT_f16[ih], -IS * ih, axis=0) if ih else adjT_f16[ih],
            "ident": ident,
        })
    return in_maps


def kernel(x, adj, W, a, trace=False, **trace_kw):
    if "nc" not in _cached:
        _cached["nc"] = build_program()
    nc = _cached["nc"]
    in_maps = make_in_maps(x, adj, W, a)
    res = run_bass_kernel_spmd(nc, in_maps, core_ids=list(range(8)),
                               trace=trace, **trace_kw)
    out = np.empty((N, 512), np.float32)
    for c in range(8):
        ih, hp = c // 4, c % 4
        o = res.results[c]["out"]           # [IS, 128] = [i_local, (k, d)]
        out[IS * ih:IS * (ih + 1), 128 * hp:128 * (hp + 1)] = o
    if trace:
        kernel.last_result = res
    return out
